# revision 20
# baseline (speedup 1.0000x reference)
"""PointMultiGraspNet-V3 segment_reduce kernel for 8 Trainium2 NeuronCores.

Strategy: channel-parallel segment-max.  feat is transposed on host to
(C=1024, N=65536); core d owns channels [128d, 128d+128) for ALL points, so
the per-segment reduce ranges (derived from `offsets` at build time) are
identical on every core and the SPMD program is fully static.  Each core
streams its (128, N) slice from HBM and reduces each segment along the free
dim -- the kernel is HBM-bandwidth bound (32 MiB/core).

The MLP head needs all 1024 channels, so each core computes its partial
features @ W1.T (contraction over its own 128 channels) and the partials are
AllReduce-summed.  The segment dim is split into uneven stages (the last one
small); each stage's partial matmul + AllReduce is issued as soon as the
stream passes its last point, so all but the last (small) collective overlap
the remaining streaming, and the first segment-block's MLP chain is emitted
mid-stream so it overlaps the stream tail too.  The rest of the MLP is
replicated on every core.  All broadcast/bias/weight constants are
host-concatenated into two contiguous tensors loaded with two line-rate DMAs.
`features` is assembled on host from the per-core channel slices;
pred/offset are taken from core 0.
"""

from bisect import bisect_right
from contextlib import ExitStack

import numpy as np

import concourse.bacc as bacc
import concourse.tile as tile
from concourse import mybir
from concourse.bass_utils import run_bass_kernel_spmd
from concourse.masks import make_identity

NCORES = 8
P = 128            # SBUF partitions == channels per core
EPS = 1e-5
KPAD = 512         # padded slots per segment in the original model
NEG = -3.0e38      # -inf stand-in (finite so 0*x etc. stay finite)
FP = mybir.dt.float32
AXX = mybir.AxisListType.X
MAX = mybir.AluOpType.max
ADD = mybir.AluOpType.add
SUB = mybir.AluOpType.subtract
MUL = mybir.AluOpType.mult
ACT_COPY = mybir.ActivationFunctionType.Copy

# bias-concat column layout (all replicated to 128 partitions on host)
_BIAS_SEGS = [("floor", 256), ("b1", 512), ("ln1g", 512), ("ln1b", 512),
              ("bi", 32), ("ba1", 256), ("lnag", 256), ("lnab", 256),
              ("ba2", 6), ("bo1", 256), ("lnog", 256), ("lnob", 256),
              ("bo2", 18)]
NB = sum(n for _, n in _BIAS_SEGS)
# weight-concat column layout (128-partition tiles)
_W_SEGS = [("w1t", 512), ("wa1_0", 256), ("wa1_1", 256), ("wa1_2", 256),
           ("wa1_3", 256), ("wo1_0", 256), ("wo1_1", 256), ("wo1_2", 256),
           ("wo1_3", 256), ("wa2_0", 6), ("wa2_1", 6), ("wo2_0", 18),
           ("wo2_1", 18)]
WC = sum(n for _, n in _W_SEGS)


def _col_ranges(segs):
    out, o = {}, 0
    for k, n in segs:
        out[k] = (o, o + n)
        o += n
    return out


_BIAS_COL = _col_ranges(_BIAS_SEGS)
_W_COL = _col_ranges(_W_SEGS)


def _stage_cuts(B):
    if B == 256:
        return [0, 64, 128, 192, 240, 256]
    step = max(1, B // 4)
    cuts = list(range(0, B, step)) + [B]
    return sorted(set(cuts))


def _build_program(bounds, n_points, cw, gb_id):
    """Build the SPMD Bass program.

    bounds: sequence of B+1 ints, bounds[0] == 0, bounds[-1] == n_points;
            segment s covers points [bounds[s], bounds[s+1]).
    gb_id: (ln1, lna, lno) -- True when that LayerNorm's gamma/beta are
           exactly ones/zeros, allowing the scale/shift ops to be skipped.
    """
    B = len(bounds) - 1
    assert B % 128 == 0
    SB = B // 128                  # 128-segment blocks (2 for B=256)
    cuts = _stage_cuts(B)
    NST = len(cuts) - 1

    nc = bacc.Bacc("TRN2", target_bir_lowering=False, debug=False,
                   num_devices=NCORES)

    # ---- DRAM I/O -------------------------------------------------------
    featT = nc.dram_tensor("featT", [P, n_points], FP, kind="ExternalInput")
    biases_d = nc.dram_tensor("biases", [P, NB], FP, kind="ExternalInput")
    wcat_d = nc.dram_tensor("wcat", [P, WC], FP, kind="ExternalInput")
    infoT_d = nc.dram_tensor("infox", [4, B], FP, kind="ExternalInput")
    wit_d = nc.dram_tensor("witx", [4, 32], FP, kind="ExternalInput")
    wa1x_d = nc.dram_tensor("wa1x", [32, 256], FP, kind="ExternalInput")
    wo1x_d = nc.dram_tensor("wo1x", [32, 256], FP, kind="ExternalInput")

    featTout_d = nc.dram_tensor("featT_out", [P, B], FP, kind="ExternalOutput")
    pred_d = nc.dram_tensor("pred_out", [B, 6], FP, kind="ExternalOutput")
    off_d = nc.dram_tensor("off_out", [B, 18], FP, kind="ExternalOutput")

    nchunks = (n_points + cw - 1) // cw
    # chunk index after which stage i's segments are complete
    st_done_chunk = [min((bounds[cuts[i + 1]] - 1) // cw, nchunks - 1) if
                     bounds[cuts[i + 1]] > 0 else 0 for i in range(NST)]

    with ExitStack() as ctx:
        tc = ctx.enter_context(tile.TileContext(nc))
        singles = ctx.enter_context(tc.tile_pool(name="singles", bufs=1))
        chunks = ctx.enter_context(tc.tile_pool(name="chunks", bufs=3))
        small = ctx.enter_context(tc.tile_pool(name="small", bufs=2))
        psum = ctx.enter_context(tc.tile_pool(name="psum", bufs=1, space="PSUM"))
        dram = ctx.enter_context(tc.tile_pool(name="dram", bufs=1, space="DRAM"))

        # ---- constant tiles (DMAs issued after chunk 1, see stream loop) -
        biases = singles.tile([P, NB], FP, tag="biases")
        wcat = singles.tile([P, WC], FP, tag="wcat")
        infoT = singles.tile([4, B], FP, tag="infoT")
        wit = singles.tile([4, 32], FP, tag="wit")
        wa1x = singles.tile([32, 256], FP, tag="wa1x")
        wo1x = singles.tile([32, 256], FP, tag="wo1x")

        def load_consts():
            nc.scalar.dma_start(out=biases[:], in_=biases_d[:, :])
            nc.scalar.dma_start(out=wcat[:], in_=wcat_d[:, :])
            nc.scalar.dma_start(out=infoT[:], in_=infoT_d[:, :])
            nc.scalar.dma_start(out=wit[:], in_=wit_d[:, :])
            nc.scalar.dma_start(out=wa1x[:], in_=wa1x_d[:, :])
            nc.scalar.dma_start(out=wo1x[:], in_=wo1x_d[:, :])

        def bia(key):
            lo, hi = _BIAS_COL[key]
            return biases[:, lo:hi]

        def wc(key):
            lo, hi = _W_COL[key]
            return wcat[:, lo:hi]

        w1sb = wc("w1t")
        wa1s = [wc(f"wa1_{c}") for c in range(4)] + [wa1x[:]]
        wo1s = [wc(f"wo1_{c}") for c in range(4)] + [wo1x[:]]
        wa2s = [wc("wa2_0"), wc("wa2_1")]
        wo2s = [wc("wo2_0"), wc("wo2_1")]
        b1b, bib = bia("b1"), bia("bi")
        ba1b, ba2b = bia("ba1"), bia("ba2")
        bo1b, bo2b = bia("bo1"), bia("bo2")
        g1b = None if gb_id[0] else bia("ln1g")
        be1b = None if gb_id[0] else bia("ln1b")
        gab = None if gb_id[1] else bia("lnag")
        beab = None if gb_id[1] else bia("lnab")
        gob = None if gb_id[2] else bia("lnog")
        beob = None if gb_id[2] else bia("lnob")

        # ---- persistent tiles -------------------------------------------
        acc_st = []
        for i in range(NST):
            a = singles.tile([P, cuts[i + 1] - cuts[i]], FP, tag=f"acc{i}")
            nc.vector.memset(a[:], NEG)
            acc_st.append(a)

        ident = singles.tile([128, 128], FP, tag="ident")
        make_identity(nc, ident[:])
        eps_t = singles.tile([128, 1], FP, tag="eps")
        nc.vector.memset(eps_t[:], float(EPS))

        ar_in_all = dram.tile([B, 512], FP, name="ar_in_all", tag="ar_in_all")
        ar_out_all = dram.tile([B, 512], FP, name="ar_out_all",
                               tag="ar_out_all")

        def seg_reduce(t, la, lb, s, first):
            """max-reduce chunk-tile columns [la,lb) into segment s's slot."""
            i = bisect_right(cuts, s) - 1
            acc_col = acc_st[i][:, s - cuts[i]:s - cuts[i] + 1]
            if first:
                nc.vector.reduce_max(out=acc_col, in_=t[:, la:lb], axis=AXX)
            else:
                tmp = small.tile([P, 1], FP, tag="tmp")
                nc.vector.reduce_max(out=tmp[:], in_=t[:, la:lb], axis=AXX)
                nc.vector.tensor_max(out=acc_col, in0=acc_col, in1=tmp[:])

        def stage_tail(i):
            """fT_i = clamp(acc_i); partial W1 matmul; AllReduce stage i."""
            ctx2 = tc.high_priority()
            ctx2.__enter__()
            lo, hi = cuts[i], cuts[i + 1]
            n = hi - lo
            fTq = singles.tile([P, n], FP, tag=f"fT{i}")
            nc.vector.tensor_max(out=fTq[:], in0=acc_st[i][:],
                                 in1=bia("floor")[:, lo:hi])
            nc.sync.dma_start(out=featTout_d[:, lo:hi], in_=fTq[:])
            p1 = psum.tile([n, 512], FP, tag="p1")
            nc.tensor.matmul(p1[:], lhsT=fTq[:], rhs=w1sb,
                             start=True, stop=True)
            cp = small.tile([n, 512], FP, tag="cp")
            if i < NST - 1:
                nc.scalar.activation(out=cp[:], in_=p1[:], func=ACT_COPY)
            else:
                nc.vector.tensor_copy(out=cp[:], in_=p1[:])
            nc.sync.dma_start(out=ar_in_all[lo:hi, :], in_=cp[:])
            nc.gpsimd.collective_compute(
                "AllReduce", ADD, replica_groups=[list(range(NCORES))],
                ins=[ar_in_all[lo:hi, :].opt()],
                outs=[ar_out_all[lo:hi, :].opt()])
            ctx2.__exit__(None, None, None)

        def layernorm_relu(out_ap, x, g, be, n, tag):
            """out_ap = relu(LN(x) * g + be); x is (128, n) sbuf."""
            st6 = small.tile([128, 6], FP, tag=f"st6_{tag}")
            nc.vector.bn_stats(out=st6[:], in_=x[:])
            mv = small.tile([128, 2], FP, tag=f"mv_{tag}")
            nc.vector.bn_aggr(out=mv[:], in_=st6[:])
            std = small.tile([128, 1], FP, tag=f"std_{tag}")
            nc.scalar.activation(out=std[:], in_=mv[:, 1:2],
                                 func=mybir.ActivationFunctionType.Sqrt,
                                 bias=eps_t[:])
            rstd = small.tile([128, 1], FP, tag=f"rstd_{tag}")
            nc.vector.reciprocal(out=rstd[:], in_=std[:])
            xh = small.tile([128, n], FP, tag=f"xh_{tag}")
            nc.vector.tensor_scalar(out=xh[:], in0=x[:], scalar1=mv[:, 0:1],
                                    scalar2=rstd[:], op0=SUB, op1=MUL)
            if g is None:
                nc.vector.tensor_scalar_max(out=out_ap, in0=xh[:], scalar1=0.0)
            else:
                nc.vector.tensor_tensor(out=xh[:], in0=xh[:], in1=g, op=MUL)
                nc.vector.tensor_tensor(out=xh[:], in0=xh[:], in1=be, op=ADD)
                nc.vector.tensor_relu(out=out_ap, in_=xh[:])

        def mlp_block(sb):
            s0 = sb * 128
            # x = concat(relu(LN(features @ W1.T + b1)), info @ Wi.T + bi)
            x1 = small.tile([128, 512], FP, tag="x1")
            for i in range(NST):
                lo, hi = max(cuts[i], s0), min(cuts[i + 1], s0 + 128)
                if hi <= lo:
                    continue
                nc.gpsimd.dma_start(out=x1[lo - s0:hi - s0, :],
                                    in_=ar_out_all[lo:hi, :])
            nc.vector.tensor_tensor(out=x1[:], in0=x1[:], in1=b1b, op=ADD)
            x_sb = singles.tile([128, 544], FP, tag=f"x_{sb}")
            layernorm_relu(x_sb[:, 0:512], x1, g1b, be1b, 512, f"pf{sb}")

            pinf = psum.tile([128, 32], FP, tag="pinf")
            nc.tensor.matmul(pinf[:], lhsT=infoT[:, s0:s0 + 128], rhs=wit[:],
                             start=True, stop=True)
            nc.vector.tensor_tensor(out=x_sb[:, 512:544], in0=pinf[:],
                                    in1=bib, op=ADD)

            # xT chunks for the 544-contraction matmuls
            xTs = []
            for c in range(5):
                w = 128 if c < 4 else 32
                pt = psum.tile([w, 128], FP, tag="pt")
                nc.tensor.transpose(out=pt[:], in_=x_sb[:, c * 128:c * 128 + w],
                                    identity=ident[:])
                xT_c = singles.tile([w, 128], FP, tag=f"xT_{sb}_{c}")
                nc.vector.tensor_copy(out=xT_c[:], in_=pt[:])
                xTs.append(xT_c)

            def head(w1tiles, bb, g, be, w2tiles, b2b, ncols, out_dram, tag):
                ph = psum.tile([128, 256], FP, tag="ph")
                for c in range(5):
                    nc.tensor.matmul(ph[:], lhsT=xTs[c][:], rhs=w1tiles[c],
                                     start=(c == 0), stop=(c == 4))
                h = small.tile([128, 256], FP, tag=f"h_{tag}")
                nc.vector.tensor_tensor(out=h[:], in0=ph[:], in1=bb, op=ADD)
                hr = small.tile([128, 256], FP, tag=f"hr_{tag}")
                layernorm_relu(hr[:], h, g, be, 256, f"h{tag}{sb}")
                pp = psum.tile([128, 32], FP, tag="pp")
                for c in range(2):
                    pt2 = psum.tile([128, 128], FP, tag="pt")
                    nc.tensor.transpose(out=pt2[:],
                                        in_=hr[:, c * 128:(c + 1) * 128],
                                        identity=ident[:])
                    hT = small.tile([128, 128], FP, tag=f"hT_{tag}")
                    nc.vector.tensor_copy(out=hT[:], in_=pt2[:])
                    nc.tensor.matmul(pp[:, 0:ncols], lhsT=hT[:],
                                     rhs=w2tiles[c],
                                     start=(c == 0), stop=(c == 1))
                outt = small.tile([128, ncols], FP, tag=f"o_{tag}")
                nc.vector.tensor_tensor(out=outt[:], in0=pp[:, 0:ncols],
                                        in1=b2b, op=ADD)
                nc.gpsimd.dma_start(out=out_dram[s0:s0 + 128, :], in_=outt[:])

            head(wa1s, ba1b, gab, beab, wa2s, ba2b, 6, pred_d, "a")
            head(wo1s, bo1b, gob, beob, wo2s, bo2b, 18, off_d, "o")

        # number of stages fully covering each seg-block
        st_cover = [sum(1 for i in range(NST) if cuts[i + 1] <= (sb + 1) * 128)
                    for sb in range(SB)]

        # ---- stream + segment reduce + interleaved stage/MLP emission ---
        stage_i = 0
        mlp_done = 0
        for ci in range(nchunks):
            c0 = ci * cw
            c1 = min(n_points, c0 + cw)
            w = c1 - c0
            t = chunks.tile([P, cw], FP, tag="chunk")
            eng = nc.sync if ci % 2 == 0 else nc.scalar
            eng.dma_start(out=t[:, :w], in_=featT[:, c0:c1])
            if ci == 1 or (nchunks == 1 and ci == 0):
                load_consts()
            for s in range(B):
                a = max(bounds[s], c0)
                b = min(bounds[s + 1], c1)
                if b <= a:
                    continue
                seg_reduce(t, a - c0, b - c0, s, bounds[s] >= c0)
            while stage_i < NST and st_done_chunk[stage_i] == ci:
                stage_tail(stage_i)
                stage_i += 1
            # emit seg-block sb's MLP once one further stage is also done:
            # its collectives are then long finished, so the chain never
            # stalls the in-order engine queues mid-stream.
            while mlp_done < SB and stage_i >= st_cover[mlp_done] + 1:
                mlp_block(mlp_done)
                mlp_done += 1
        while stage_i < NST:    # stages with no points still need output
            stage_tail(stage_i)
            stage_i += 1
        while mlp_done < SB:
            mlp_block(mlp_done)
            mlp_done += 1

    nc.compile()
    return nc


_PROG_CACHE = {}

# test harness hooks: set TRACE=True before calling kernel() to capture an
# NTFF profile; the measured NEFF time lands in LAST_EXEC_NS.
TRACE = False
LAST_EXEC_NS = None
LAST_RESULTS = None


def _ensure_ntff_hook():
    """The image's antenv package lacks axon_hooks; synthesize it so
    run_bass_kernel_spmd(trace=True) can reach the NTFF profiler."""
    import sys
    import types
    try:
        from antenv.axon_hooks import get_axon_ntff_profile_hook  # noqa: F401
        return
    except ImportError:
        pass
    import antenv
    from trn_agent_boot.trn_boot import _ntff_profile_via_ctypes
    hook = _ntff_profile_via_ctypes("/opt/axon/libaxon_pjrt.so")
    m = types.ModuleType("antenv.axon_hooks")
    m.get_axon_ntff_profile_hook = lambda: hook
    m.set_axon_ntff_profile_hook = lambda h: None
    sys.modules["antenv.axon_hooks"] = m
    antenv.axon_hooks = m


def _get_program(bounds_t, n_points, cw, gb_id):
    key = (bounds_t, n_points, cw, gb_id)
    if key not in _PROG_CACHE:
        _PROG_CACHE[key] = _build_program(list(bounds_t), n_points, cw, gb_id)
    return _PROG_CACHE[key]


def _make_in_maps(feat, info, offsets, wd, B):
    """Build the 8 per-core input maps. wd: dict of weight arrays."""
    featT = np.ascontiguousarray(feat.T)                      # (C, N)
    w1T = np.ascontiguousarray(wd["W1"].T)                    # (C, 512)
    lens = np.diff(offsets, prepend=0)
    floor = np.where(lens < KPAD, 0.0, NEG).astype(np.float32)

    vals = dict(
        floor=floor, b1=wd["b1"], ln1g=wd["ln1_g"], ln1b=wd["ln1_b"],
        bi=wd["bi"], ba1=wd["ba1"], lnag=wd["lna_g"], lnab=wd["lna_b"],
        ba2=wd["ba2"], bo1=wd["bo1"], lnog=wd["lno_g"], lnob=wd["lno_b"],
        bo2=wd["bo2"])
    brow = np.concatenate([np.asarray(vals[k], np.float32).ravel()
                           for k, _ in _BIAS_SEGS])
    assert brow.shape[0] == NB
    biases = np.ascontiguousarray(np.broadcast_to(brow, (P, NB)))

    wa1T = np.ascontiguousarray(wd["Wa1"].T)                  # (544, 256)
    wo1T = np.ascontiguousarray(wd["Wo1"].T)
    wa2T = np.ascontiguousarray(wd["Wa2"].T)                  # (256, 6)
    wo2T = np.ascontiguousarray(wd["Wo2"].T)                  # (256, 18)
    infoT = np.zeros((4, B), np.float32)
    infoT[:3] = info.T
    wit = np.zeros((4, 32), np.float32)
    wit[:3] = wd["Wi"].T

    def wcat_for(w1t_slice):
        parts = [w1t_slice]
        parts += [wa1T[c * 128:(c + 1) * 128] for c in range(4)]
        parts += [wo1T[c * 128:(c + 1) * 128] for c in range(4)]
        parts += [wa2T[0:128], wa2T[128:256]]
        parts += [wo2T[0:128], wo2T[128:256]]
        return np.ascontiguousarray(np.concatenate(parts, axis=1))

    common = dict(
        biases=biases,
        infox=infoT, witx=wit,
        wa1x=np.ascontiguousarray(wa1T[512:544]),
        wo1x=np.ascontiguousarray(wo1T[512:544]),
    )
    common = {k: np.ascontiguousarray(v, dtype=np.float32)
              for k, v in common.items()}
    in_maps = []
    for d in range(NCORES):
        m = dict(common)
        m["featT"] = featT[d * P:(d + 1) * P]
        m["wcat"] = wcat_for(w1T[d * P:(d + 1) * P])
        in_maps.append(m)
    return in_maps


def _gb_identity(wd):
    def iden(g, b):
        return bool(np.all(np.asarray(g) == 1.0) and
                    np.all(np.asarray(b) == 0.0))
    return (iden(wd["ln1_g"], wd["ln1_b"]),
            iden(wd["lna_g"], wd["lna_b"]),
            iden(wd["lno_g"], wd["lno_b"]))


def kernel(**inputs):
    xs = {k: np.asarray(v) for k, v in inputs.items()}
    feat = np.ascontiguousarray(xs["feat"], dtype=np.float32)
    info = np.ascontiguousarray(xs["info"], dtype=np.float32)
    offsets = np.asarray(xs["offsets"]).astype(np.int64)
    n, c = feat.shape
    B = offsets.shape[0]
    assert c == NCORES * P

    bounds = np.concatenate([[0], offsets]).astype(np.int64)
    cw = 8192
    gb_id = _gb_identity(xs)
    nc = _get_program(tuple(int(v) for v in bounds), n, cw, gb_id)
    in_maps = _make_in_maps(feat, info, offsets, xs, B)

    if TRACE:
        _ensure_ntff_hook()
        import concourse.bass_utils as _bu
        _bu.upload_artifacts = lambda d: d  # no S3 in this container
    res = run_bass_kernel_spmd(nc, in_maps, core_ids=list(range(NCORES)),
                               trace=TRACE)
    global LAST_EXEC_NS, LAST_RESULTS
    LAST_EXEC_NS = res.exec_time_ns
    LAST_RESULTS = res
    featuresT = np.concatenate(
        [res.results[d]["featT_out"] for d in range(NCORES)], axis=0)  # (C, B)
    features = np.ascontiguousarray(featuresT.T)
    pred = res.results[0]["pred_out"]
    offset = res.results[0]["off_out"].reshape(B, 6, 3)
    return features, pred, offset


# revision 22
# speedup vs baseline: 1.0209x; 1.0209x over previous
"""PointMultiGraspNet-V3 segment_reduce kernel for 8 Trainium2 NeuronCores.

Strategy: channel-parallel segment-max.  feat is transposed on host to
(C=1024, N=65536); core d owns channels [128d, 128d+128) for ALL points, so
the per-segment reduce ranges (derived from `offsets` at build time) are
identical on every core and the SPMD program is fully static.  Each core
streams its (128, N) slice from HBM and reduces each segment along the free
dim -- the kernel is HBM-bandwidth bound (32 MiB/core).

The MLP head needs all 1024 channels, so each core computes its partial
features @ W1.T (contraction over its own 128 channels) and the partials are
AllReduce-summed.  The segment dim is split into uneven stages (the last one
small); each stage's partial matmul + AllReduce is issued as soon as the
stream passes its last point, so all but the last (small) collective overlap
the remaining streaming, and the first segment-block's MLP chain is emitted
mid-stream so it overlaps the stream tail too.  The rest of the MLP is
replicated on every core.  All broadcast/bias/weight constants are
host-concatenated into two contiguous tensors loaded with two line-rate DMAs.
`features` is assembled on host from the per-core channel slices;
pred/offset are taken from core 0.
"""

from bisect import bisect_right
from contextlib import ExitStack

import numpy as np

import concourse.bacc as bacc
import concourse.tile as tile
from concourse import mybir
from concourse.bass_utils import run_bass_kernel_spmd
from concourse.masks import make_identity

NCORES = 8
P = 128            # SBUF partitions == channels per core
EPS = 1e-5
KPAD = 512         # padded slots per segment in the original model
NEG = -3.0e38      # -inf stand-in (finite so 0*x etc. stay finite)
FP = mybir.dt.float32
AXX = mybir.AxisListType.X
MAX = mybir.AluOpType.max
ADD = mybir.AluOpType.add
SUB = mybir.AluOpType.subtract
MUL = mybir.AluOpType.mult
ACT_COPY = mybir.ActivationFunctionType.Copy

# bias-concat column layout (all replicated to 128 partitions on host)
_BIAS_SEGS = [("floor", 256), ("b1", 512), ("ln1g", 512), ("ln1b", 512),
              ("bi", 32), ("ba1", 256), ("lnag", 256), ("lnab", 256),
              ("ba2", 6), ("bo1", 256), ("lnog", 256), ("lnob", 256),
              ("bo2", 18)]
NB = sum(n for _, n in _BIAS_SEGS)
# weight-concat column layout (128-partition tiles)
_W_SEGS = [("w1t", 512), ("wa1_0", 256), ("wa1_1", 256), ("wa1_2", 256),
           ("wa1_3", 256), ("wo1_0", 256), ("wo1_1", 256), ("wo1_2", 256),
           ("wo1_3", 256), ("wa2_0", 6), ("wa2_1", 6), ("wo2_0", 18),
           ("wo2_1", 18)]
WC = sum(n for _, n in _W_SEGS)


def _col_ranges(segs):
    out, o = {}, 0
    for k, n in segs:
        out[k] = (o, o + n)
        o += n
    return out


_BIAS_COL = _col_ranges(_BIAS_SEGS)
_W_COL = _col_ranges(_W_SEGS)


def _stage_cuts(B):
    if B == 256:
        return [0, 64, 128, 192, 240, 256]
    step = max(1, B // 4)
    cuts = list(range(0, B, step)) + [B]
    return sorted(set(cuts))


def _build_program(bounds, n_points, cw, gb_id):
    """Build the SPMD Bass program.

    bounds: sequence of B+1 ints, bounds[0] == 0, bounds[-1] == n_points;
            segment s covers points [bounds[s], bounds[s+1]).
    gb_id: (ln1, lna, lno) -- True when that LayerNorm's gamma/beta are
           exactly ones/zeros, allowing the scale/shift ops to be skipped.
    """
    B = len(bounds) - 1
    assert B % 128 == 0
    SB = B // 128                  # 128-segment blocks (2 for B=256)
    cuts = _stage_cuts(B)
    NST = len(cuts) - 1

    nc = bacc.Bacc("TRN2", target_bir_lowering=False, debug=False,
                   num_devices=NCORES)

    # ---- DRAM I/O -------------------------------------------------------
    featT = nc.dram_tensor("featT", [P, n_points], FP, kind="ExternalInput")
    biases_d = nc.dram_tensor("biases", [P, NB], FP, kind="ExternalInput")
    wcat_d = nc.dram_tensor("wcat", [P, WC], FP, kind="ExternalInput")
    infoT_d = nc.dram_tensor("infox", [4, B], FP, kind="ExternalInput")
    wit_d = nc.dram_tensor("witx", [4, 32], FP, kind="ExternalInput")
    wa1x_d = nc.dram_tensor("wa1x", [32, 256], FP, kind="ExternalInput")
    wo1x_d = nc.dram_tensor("wo1x", [32, 256], FP, kind="ExternalInput")

    featTout_d = nc.dram_tensor("featT_out", [P, B], FP, kind="ExternalOutput")
    pred_d = nc.dram_tensor("pred_out", [B, 6], FP, kind="ExternalOutput")
    off_d = nc.dram_tensor("off_out", [B, 18], FP, kind="ExternalOutput")

    nchunks = (n_points + cw - 1) // cw
    # chunk index after which stage i's segments are complete
    st_done_chunk = [min((bounds[cuts[i + 1]] - 1) // cw, nchunks - 1) if
                     bounds[cuts[i + 1]] > 0 else 0 for i in range(NST)]

    with ExitStack() as ctx:
        tc = ctx.enter_context(tile.TileContext(nc))
        singles = ctx.enter_context(tc.tile_pool(name="singles", bufs=1))
        chunks = ctx.enter_context(tc.tile_pool(name="chunks", bufs=3))
        small = ctx.enter_context(tc.tile_pool(name="small", bufs=2))
        psum = ctx.enter_context(tc.tile_pool(name="psum", bufs=1, space="PSUM"))
        dram = ctx.enter_context(tc.tile_pool(name="dram", bufs=1, space="DRAM"))

        # ---- constant tiles (DMAs issued after chunk 1, see stream loop) -
        biases = singles.tile([P, NB], FP, tag="biases")
        wcat = singles.tile([P, WC], FP, tag="wcat")
        infoT = singles.tile([4, B], FP, tag="infoT")
        wit = singles.tile([4, 32], FP, tag="wit")
        wa1x = singles.tile([32, 256], FP, tag="wa1x")
        wo1x = singles.tile([32, 256], FP, tag="wo1x")

        def load_consts():
            nc.scalar.dma_start(out=biases[:], in_=biases_d[:, :])
            nc.scalar.dma_start(out=wcat[:], in_=wcat_d[:, :])
            nc.scalar.dma_start(out=infoT[:], in_=infoT_d[:, :])
            nc.scalar.dma_start(out=wit[:], in_=wit_d[:, :])
            nc.scalar.dma_start(out=wa1x[:], in_=wa1x_d[:, :])
            nc.scalar.dma_start(out=wo1x[:], in_=wo1x_d[:, :])

        def bia(key):
            lo, hi = _BIAS_COL[key]
            return biases[:, lo:hi]

        def wc(key):
            lo, hi = _W_COL[key]
            return wcat[:, lo:hi]

        w1sb = wc("w1t")
        wa1s = [wc(f"wa1_{c}") for c in range(4)] + [wa1x[:]]
        wo1s = [wc(f"wo1_{c}") for c in range(4)] + [wo1x[:]]
        wa2s = [wc("wa2_0"), wc("wa2_1")]
        wo2s = [wc("wo2_0"), wc("wo2_1")]
        b1b, bib = bia("b1"), bia("bi")
        ba1b, ba2b = bia("ba1"), bia("ba2")
        bo1b, bo2b = bia("bo1"), bia("bo2")
        g1b = None if gb_id[0] else bia("ln1g")
        be1b = None if gb_id[0] else bia("ln1b")
        gab = None if gb_id[1] else bia("lnag")
        beab = None if gb_id[1] else bia("lnab")
        gob = None if gb_id[2] else bia("lnog")
        beob = None if gb_id[2] else bia("lnob")

        # ---- persistent tiles -------------------------------------------
        acc_st = []
        for i in range(NST):
            a = singles.tile([P, cuts[i + 1] - cuts[i]], FP, tag=f"acc{i}")
            nc.vector.memset(a[:], NEG)
            acc_st.append(a)

        ident = singles.tile([128, 128], FP, tag="ident")
        make_identity(nc, ident[:])
        eps_t = singles.tile([128, 1], FP, tag="eps")
        nc.vector.memset(eps_t[:], float(EPS))

        # dummy collective issued at t~0: absorbs the expensive first-use
        # warm-up of the collective path while the stream runs.
        warm = dram.tile([1, 128], FP, name="warm_in", tag="warm_in")
        warm_o = dram.tile([1, 128], FP, name="warm_out", tag="warm_out")
        warm_sb = singles.tile([1, 128], FP, tag="warm_sb")
        nc.vector.memset(warm_sb[:], 0.0)
        nc.sync.dma_start(out=warm[:], in_=warm_sb[:])
        nc.gpsimd.collective_compute(
            "AllReduce", ADD, replica_groups=[list(range(NCORES))],
            ins=[warm[:].opt()], outs=[warm_o[:].opt()])

        ar_in_all = dram.tile([B, 512], FP, name="ar_in_all", tag="ar_in_all")
        ar_out_all = dram.tile([B, 512], FP, name="ar_out_all",
                               tag="ar_out_all")

        def seg_reduce(t, la, lb, s, first):
            """max-reduce chunk-tile columns [la,lb) into segment s's slot."""
            i = bisect_right(cuts, s) - 1
            acc_col = acc_st[i][:, s - cuts[i]:s - cuts[i] + 1]
            if first:
                nc.vector.reduce_max(out=acc_col, in_=t[:, la:lb], axis=AXX)
            else:
                tmp = small.tile([P, 1], FP, tag="tmp")
                nc.vector.reduce_max(out=tmp[:], in_=t[:, la:lb], axis=AXX)
                nc.vector.tensor_max(out=acc_col, in0=acc_col, in1=tmp[:])

        def stage_tail(i):
            """fT_i = clamp(acc_i); partial W1 matmul; AllReduce stage i."""
            lo, hi = cuts[i], cuts[i + 1]
            n = hi - lo
            fTq = singles.tile([P, n], FP, tag=f"fT{i}")
            nc.vector.tensor_max(out=fTq[:], in0=acc_st[i][:],
                                 in1=bia("floor")[:, lo:hi])
            nc.sync.dma_start(out=featTout_d[:, lo:hi], in_=fTq[:])
            p1 = psum.tile([n, 512], FP, tag="p1")
            nc.tensor.matmul(p1[:], lhsT=fTq[:], rhs=w1sb,
                             start=True, stop=True)
            cp = small.tile([n, 512], FP, tag="cp")
            if i < NST - 1:
                nc.scalar.activation(out=cp[:], in_=p1[:], func=ACT_COPY)
            else:
                nc.vector.tensor_copy(out=cp[:], in_=p1[:])
            nc.sync.dma_start(out=ar_in_all[lo:hi, :], in_=cp[:])
            nc.gpsimd.collective_compute(
                "AllReduce", ADD, replica_groups=[list(range(NCORES))],
                ins=[ar_in_all[lo:hi, :].opt()],
                outs=[ar_out_all[lo:hi, :].opt()])

        def layernorm_relu(out_ap, x, g, be, n, tag):
            """out_ap = relu(LN(x) * g + be); x is (128, n) sbuf."""
            st6 = small.tile([128, 6], FP, tag=f"st6_{tag}")
            nc.vector.bn_stats(out=st6[:], in_=x[:])
            mv = small.tile([128, 2], FP, tag=f"mv_{tag}")
            nc.vector.bn_aggr(out=mv[:], in_=st6[:])
            std = small.tile([128, 1], FP, tag=f"std_{tag}")
            nc.scalar.activation(out=std[:], in_=mv[:, 1:2],
                                 func=mybir.ActivationFunctionType.Sqrt,
                                 bias=eps_t[:])
            rstd = small.tile([128, 1], FP, tag=f"rstd_{tag}")
            nc.vector.reciprocal(out=rstd[:], in_=std[:])
            xh = small.tile([128, n], FP, tag=f"xh_{tag}")
            nc.vector.tensor_scalar(out=xh[:], in0=x[:], scalar1=mv[:, 0:1],
                                    scalar2=rstd[:], op0=SUB, op1=MUL)
            if g is None:
                nc.vector.tensor_scalar_max(out=out_ap, in0=xh[:], scalar1=0.0)
            else:
                nc.vector.tensor_tensor(out=xh[:], in0=xh[:], in1=g, op=MUL)
                nc.vector.tensor_tensor(out=xh[:], in0=xh[:], in1=be, op=ADD)
                nc.vector.tensor_relu(out=out_ap, in_=xh[:])

        def mlp_block(sb):
            s0 = sb * 128
            # x = concat(relu(LN(features @ W1.T + b1)), info @ Wi.T + bi)
            x1 = small.tile([128, 512], FP, tag="x1")
            for i in range(NST):
                lo, hi = max(cuts[i], s0), min(cuts[i + 1], s0 + 128)
                if hi <= lo:
                    continue
                nc.gpsimd.dma_start(out=x1[lo - s0:hi - s0, :],
                                    in_=ar_out_all[lo:hi, :])
            nc.vector.tensor_tensor(out=x1[:], in0=x1[:], in1=b1b, op=ADD)
            x_sb = singles.tile([128, 544], FP, tag=f"x_{sb}")
            layernorm_relu(x_sb[:, 0:512], x1, g1b, be1b, 512, f"pf{sb}")

            pinf = psum.tile([128, 32], FP, tag="pinf")
            nc.tensor.matmul(pinf[:], lhsT=infoT[:, s0:s0 + 128], rhs=wit[:],
                             start=True, stop=True)
            nc.vector.tensor_tensor(out=x_sb[:, 512:544], in0=pinf[:],
                                    in1=bib, op=ADD)

            # xT chunks for the 544-contraction matmuls
            xTs = []
            for c in range(5):
                w = 128 if c < 4 else 32
                pt = psum.tile([w, 128], FP, tag="pt")
                nc.tensor.transpose(out=pt[:], in_=x_sb[:, c * 128:c * 128 + w],
                                    identity=ident[:])
                xT_c = singles.tile([w, 128], FP, tag=f"xT_{sb}_{c}")
                nc.vector.tensor_copy(out=xT_c[:], in_=pt[:])
                xTs.append(xT_c)

            def head(w1tiles, bb, g, be, w2tiles, b2b, ncols, out_dram, tag):
                ph = psum.tile([128, 256], FP, tag="ph")
                for c in range(5):
                    nc.tensor.matmul(ph[:], lhsT=xTs[c][:], rhs=w1tiles[c],
                                     start=(c == 0), stop=(c == 4))
                h = small.tile([128, 256], FP, tag=f"h_{tag}")
                nc.vector.tensor_tensor(out=h[:], in0=ph[:], in1=bb, op=ADD)
                hr = small.tile([128, 256], FP, tag=f"hr_{tag}")
                layernorm_relu(hr[:], h, g, be, 256, f"h{tag}{sb}")
                pp = psum.tile([128, 32], FP, tag="pp")
                for c in range(2):
                    pt2 = psum.tile([128, 128], FP, tag="pt")
                    nc.tensor.transpose(out=pt2[:],
                                        in_=hr[:, c * 128:(c + 1) * 128],
                                        identity=ident[:])
                    hT = small.tile([128, 128], FP, tag=f"hT_{tag}")
                    nc.vector.tensor_copy(out=hT[:], in_=pt2[:])
                    nc.tensor.matmul(pp[:, 0:ncols], lhsT=hT[:],
                                     rhs=w2tiles[c],
                                     start=(c == 0), stop=(c == 1))
                outt = small.tile([128, ncols], FP, tag=f"o_{tag}")
                nc.vector.tensor_tensor(out=outt[:], in0=pp[:, 0:ncols],
                                        in1=b2b, op=ADD)
                nc.gpsimd.dma_start(out=out_dram[s0:s0 + 128, :], in_=outt[:])

            head(wa1s, ba1b, gab, beab, wa2s, ba2b, 6, pred_d, "a")
            head(wo1s, bo1b, gob, beob, wo2s, bo2b, 18, off_d, "o")

        # number of stages fully covering each seg-block
        st_cover = [sum(1 for i in range(NST) if cuts[i + 1] <= (sb + 1) * 128)
                    for sb in range(SB)]

        # ---- stream + segment reduce + interleaved stage/MLP emission ---
        stage_i = 0
        mlp_done = 0
        for ci in range(nchunks):
            c0 = ci * cw
            c1 = min(n_points, c0 + cw)
            w = c1 - c0
            t = chunks.tile([P, cw], FP, tag="chunk")
            eng = nc.sync if ci % 2 == 0 else nc.scalar
            eng.dma_start(out=t[:, :w], in_=featT[:, c0:c1])
            if ci == 1 or (nchunks == 1 and ci == 0):
                load_consts()
            for s in range(B):
                a = max(bounds[s], c0)
                b = min(bounds[s + 1], c1)
                if b <= a:
                    continue
                seg_reduce(t, a - c0, b - c0, s, bounds[s] >= c0)
            while stage_i < NST and st_done_chunk[stage_i] == ci:
                stage_tail(stage_i)
                stage_i += 1
            # emit seg-block sb's MLP once one further stage is also done:
            # its collectives are then long finished, so the chain never
            # stalls the in-order engine queues mid-stream.
            while mlp_done < SB and stage_i >= st_cover[mlp_done] + 1:
                mlp_block(mlp_done)
                mlp_done += 1
        while stage_i < NST:    # stages with no points still need output
            stage_tail(stage_i)
            stage_i += 1
        while mlp_done < SB:
            mlp_block(mlp_done)
            mlp_done += 1

    nc.compile()
    return nc


_PROG_CACHE = {}

# test harness hooks: set TRACE=True before calling kernel() to capture an
# NTFF profile; the measured NEFF time lands in LAST_EXEC_NS.
TRACE = False
LAST_EXEC_NS = None
LAST_RESULTS = None


def _ensure_ntff_hook():
    """The image's antenv package lacks axon_hooks; synthesize it so
    run_bass_kernel_spmd(trace=True) can reach the NTFF profiler."""
    import sys
    import types
    try:
        from antenv.axon_hooks import get_axon_ntff_profile_hook  # noqa: F401
        return
    except ImportError:
        pass
    import antenv
    from trn_agent_boot.trn_boot import _ntff_profile_via_ctypes
    hook = _ntff_profile_via_ctypes("/opt/axon/libaxon_pjrt.so")
    m = types.ModuleType("antenv.axon_hooks")
    m.get_axon_ntff_profile_hook = lambda: hook
    m.set_axon_ntff_profile_hook = lambda h: None
    sys.modules["antenv.axon_hooks"] = m
    antenv.axon_hooks = m


def _get_program(bounds_t, n_points, cw, gb_id):
    key = (bounds_t, n_points, cw, gb_id)
    if key not in _PROG_CACHE:
        _PROG_CACHE[key] = _build_program(list(bounds_t), n_points, cw, gb_id)
    return _PROG_CACHE[key]


def _make_in_maps(feat, info, offsets, wd, B):
    """Build the 8 per-core input maps. wd: dict of weight arrays."""
    featT = np.ascontiguousarray(feat.T)                      # (C, N)
    w1T = np.ascontiguousarray(wd["W1"].T)                    # (C, 512)
    lens = np.diff(offsets, prepend=0)
    floor = np.where(lens < KPAD, 0.0, NEG).astype(np.float32)

    vals = dict(
        floor=floor, b1=wd["b1"], ln1g=wd["ln1_g"], ln1b=wd["ln1_b"],
        bi=wd["bi"], ba1=wd["ba1"], lnag=wd["lna_g"], lnab=wd["lna_b"],
        ba2=wd["ba2"], bo1=wd["bo1"], lnog=wd["lno_g"], lnob=wd["lno_b"],
        bo2=wd["bo2"])
    brow = np.concatenate([np.asarray(vals[k], np.float32).ravel()
                           for k, _ in _BIAS_SEGS])
    assert brow.shape[0] == NB
    biases = np.ascontiguousarray(np.broadcast_to(brow, (P, NB)))

    wa1T = np.ascontiguousarray(wd["Wa1"].T)                  # (544, 256)
    wo1T = np.ascontiguousarray(wd["Wo1"].T)
    wa2T = np.ascontiguousarray(wd["Wa2"].T)                  # (256, 6)
    wo2T = np.ascontiguousarray(wd["Wo2"].T)                  # (256, 18)
    infoT = np.zeros((4, B), np.float32)
    infoT[:3] = info.T
    wit = np.zeros((4, 32), np.float32)
    wit[:3] = wd["Wi"].T

    def wcat_for(w1t_slice):
        parts = [w1t_slice]
        parts += [wa1T[c * 128:(c + 1) * 128] for c in range(4)]
        parts += [wo1T[c * 128:(c + 1) * 128] for c in range(4)]
        parts += [wa2T[0:128], wa2T[128:256]]
        parts += [wo2T[0:128], wo2T[128:256]]
        return np.ascontiguousarray(np.concatenate(parts, axis=1))

    common = dict(
        biases=biases,
        infox=infoT, witx=wit,
        wa1x=np.ascontiguousarray(wa1T[512:544]),
        wo1x=np.ascontiguousarray(wo1T[512:544]),
    )
    common = {k: np.ascontiguousarray(v, dtype=np.float32)
              for k, v in common.items()}
    in_maps = []
    for d in range(NCORES):
        m = dict(common)
        m["featT"] = featT[d * P:(d + 1) * P]
        m["wcat"] = wcat_for(w1T[d * P:(d + 1) * P])
        in_maps.append(m)
    return in_maps


def _gb_identity(wd):
    def iden(g, b):
        return bool(np.all(np.asarray(g) == 1.0) and
                    np.all(np.asarray(b) == 0.0))
    return (iden(wd["ln1_g"], wd["ln1_b"]),
            iden(wd["lna_g"], wd["lna_b"]),
            iden(wd["lno_g"], wd["lno_b"]))


def kernel(**inputs):
    xs = {k: np.asarray(v) for k, v in inputs.items()}
    feat = np.ascontiguousarray(xs["feat"], dtype=np.float32)
    info = np.ascontiguousarray(xs["info"], dtype=np.float32)
    offsets = np.asarray(xs["offsets"]).astype(np.int64)
    n, c = feat.shape
    B = offsets.shape[0]
    assert c == NCORES * P

    bounds = np.concatenate([[0], offsets]).astype(np.int64)
    cw = 8192
    gb_id = _gb_identity(xs)
    nc = _get_program(tuple(int(v) for v in bounds), n, cw, gb_id)
    in_maps = _make_in_maps(feat, info, offsets, xs, B)

    if TRACE:
        _ensure_ntff_hook()
        import concourse.bass_utils as _bu
        _bu.upload_artifacts = lambda d: d  # no S3 in this container
    res = run_bass_kernel_spmd(nc, in_maps, core_ids=list(range(NCORES)),
                               trace=TRACE)
    global LAST_EXEC_NS, LAST_RESULTS
    LAST_EXEC_NS = res.exec_time_ns
    LAST_RESULTS = res
    featuresT = np.concatenate(
        [res.results[d]["featT_out"] for d in range(NCORES)], axis=0)  # (C, B)
    features = np.ascontiguousarray(featuresT.T)
    pred = res.results[0]["pred_out"]
    offset = res.results[0]["off_out"].reshape(B, 6, 3)
    return features, pred, offset


# revision 23
# speedup vs baseline: 1.1485x; 1.1250x over previous
"""PointMultiGraspNet-V3 segment_reduce kernel for 8 Trainium2 NeuronCores.

Strategy: channel-parallel segment-max.  feat is transposed on host to
(C=1024, N=65536); core d owns channels [128d, 128d+128) for ALL points, so
the per-segment reduce ranges (derived from `offsets` at build time) are
identical on every core and the SPMD program is fully static.  Each core
streams its (128, N) slice from HBM and reduces each segment along the free
dim -- the kernel is HBM-bandwidth bound (32 MiB/core).

The MLP head needs all 1024 channels, so each core computes its partial
features @ W1.T (contraction over its own 128 channels) and the partials are
AllReduce-summed.  The segment dim is split into uneven stages (the last one
small); each stage's partial matmul + AllReduce is issued as soon as the
stream passes its last point, so all but the last (small) collective overlap
the remaining streaming, and the first segment-block's MLP chain is emitted
mid-stream so it overlaps the stream tail too.  The rest of the MLP is
replicated on every core.  All broadcast/bias/weight constants are
host-concatenated into two contiguous tensors loaded with two line-rate DMAs.
`features` is assembled on host from the per-core channel slices;
pred/offset are taken from core 0.
"""

from bisect import bisect_right
from contextlib import ExitStack

import numpy as np

import concourse.bacc as bacc
import concourse.tile as tile
from concourse import mybir
from concourse.bass_utils import run_bass_kernel_spmd
from concourse.masks import make_identity

NCORES = 8
P = 128            # SBUF partitions == channels per core
EPS = 1e-5
KPAD = 512         # padded slots per segment in the original model
NEG = -3.0e38      # -inf stand-in (finite so 0*x etc. stay finite)
FP = mybir.dt.float32
AXX = mybir.AxisListType.X
MAX = mybir.AluOpType.max
ADD = mybir.AluOpType.add
SUB = mybir.AluOpType.subtract
MUL = mybir.AluOpType.mult
ACT_COPY = mybir.ActivationFunctionType.Copy

# bias-concat column layout (all replicated to 128 partitions on host)
_BIAS_SEGS = [("floor", 256), ("b1", 512), ("ln1g", 512), ("ln1b", 512),
              ("bi", 32), ("ba1", 256), ("lnag", 256), ("lnab", 256),
              ("ba2", 6), ("bo1", 256), ("lnog", 256), ("lnob", 256),
              ("bo2", 18)]
NB = sum(n for _, n in _BIAS_SEGS)
# weight-concat column layout (128-partition tiles)
_W_SEGS = [("w1t", 512), ("wa1_0", 256), ("wa1_1", 256), ("wa1_2", 256),
           ("wa1_3", 256), ("wo1_0", 256), ("wo1_1", 256), ("wo1_2", 256),
           ("wo1_3", 256), ("wa2_0", 6), ("wa2_1", 6), ("wo2_0", 18),
           ("wo2_1", 18)]
WC = sum(n for _, n in _W_SEGS)


def _col_ranges(segs):
    out, o = {}, 0
    for k, n in segs:
        out[k] = (o, o + n)
        o += n
    return out


_BIAS_COL = _col_ranges(_BIAS_SEGS)
_W_COL = _col_ranges(_W_SEGS)


def _stage_cuts(B):
    return [0, B // 2, B]


def _build_program(bounds, n_points, cw, gb_id):
    """Build the SPMD Bass program.

    bounds: sequence of B+1 ints, bounds[0] == 0, bounds[-1] == n_points;
            segment s covers points [bounds[s], bounds[s+1]).
    gb_id: (ln1, lna, lno) -- True when that LayerNorm's gamma/beta are
           exactly ones/zeros, allowing the scale/shift ops to be skipped.
    """
    B = len(bounds) - 1
    assert B % 128 == 0
    SB = B // 128                  # 128-segment blocks (2 for B=256)
    cuts = _stage_cuts(B)
    NST = len(cuts) - 1

    nc = bacc.Bacc("TRN2", target_bir_lowering=False, debug=False,
                   num_devices=NCORES)

    # ---- DRAM I/O -------------------------------------------------------
    featT = nc.dram_tensor("featT", [P, n_points], FP, kind="ExternalInput")
    biases_d = nc.dram_tensor("biases", [P, NB], FP, kind="ExternalInput")
    wcat_d = nc.dram_tensor("wcat", [P, WC], FP, kind="ExternalInput")
    infoT_d = nc.dram_tensor("infox", [4, B], FP, kind="ExternalInput")
    wit_d = nc.dram_tensor("witx", [4, 32], FP, kind="ExternalInput")
    wa1x_d = nc.dram_tensor("wa1x", [32, 256], FP, kind="ExternalInput")
    wo1x_d = nc.dram_tensor("wo1x", [32, 256], FP, kind="ExternalInput")

    featTout_d = nc.dram_tensor("featT_out", [P, B], FP, kind="ExternalOutput")
    pred_d = nc.dram_tensor("pred_out", [B, 6], FP, kind="ExternalOutput")
    off_d = nc.dram_tensor("off_out", [B, 18], FP, kind="ExternalOutput")

    nchunks = (n_points + cw - 1) // cw
    # chunk index after which stage i's segments are complete
    st_done_chunk = [min((bounds[cuts[i + 1]] - 1) // cw, nchunks - 1) if
                     bounds[cuts[i + 1]] > 0 else 0 for i in range(NST)]

    with ExitStack() as ctx:
        tc = ctx.enter_context(tile.TileContext(nc))
        singles = ctx.enter_context(tc.tile_pool(name="singles", bufs=1))
        chunks = ctx.enter_context(tc.tile_pool(name="chunks", bufs=3))
        small = ctx.enter_context(tc.tile_pool(name="small", bufs=2))
        psum = ctx.enter_context(tc.tile_pool(name="psum", bufs=1, space="PSUM"))
        dram = ctx.enter_context(tc.tile_pool(name="dram", bufs=1, space="DRAM"))

        # ---- constant tiles (DMAs issued after chunk 1, see stream loop) -
        biases = singles.tile([P, NB], FP, tag="biases")
        wcat = singles.tile([P, WC], FP, tag="wcat")
        infoT = singles.tile([4, B], FP, tag="infoT")
        wit = singles.tile([4, 32], FP, tag="wit")
        wa1x = singles.tile([32, 256], FP, tag="wa1x")
        wo1x = singles.tile([32, 256], FP, tag="wo1x")

        def load_consts():
            nc.scalar.dma_start(out=biases[:], in_=biases_d[:, :])
            nc.scalar.dma_start(out=wcat[:], in_=wcat_d[:, :])
            nc.scalar.dma_start(out=infoT[:], in_=infoT_d[:, :])
            nc.scalar.dma_start(out=wit[:], in_=wit_d[:, :])
            nc.scalar.dma_start(out=wa1x[:], in_=wa1x_d[:, :])
            nc.scalar.dma_start(out=wo1x[:], in_=wo1x_d[:, :])

        def bia(key):
            lo, hi = _BIAS_COL[key]
            return biases[:, lo:hi]

        def wc(key):
            lo, hi = _W_COL[key]
            return wcat[:, lo:hi]

        w1sb = wc("w1t")
        wa1s = [wc(f"wa1_{c}") for c in range(4)] + [wa1x[:]]
        wo1s = [wc(f"wo1_{c}") for c in range(4)] + [wo1x[:]]
        wa2s = [wc("wa2_0"), wc("wa2_1")]
        wo2s = [wc("wo2_0"), wc("wo2_1")]
        b1b, bib = bia("b1"), bia("bi")
        ba1b, ba2b = bia("ba1"), bia("ba2")
        bo1b, bo2b = bia("bo1"), bia("bo2")
        g1b = None if gb_id[0] else bia("ln1g")
        be1b = None if gb_id[0] else bia("ln1b")
        gab = None if gb_id[1] else bia("lnag")
        beab = None if gb_id[1] else bia("lnab")
        gob = None if gb_id[2] else bia("lnog")
        beob = None if gb_id[2] else bia("lnob")

        # ---- persistent tiles -------------------------------------------
        acc_st = []
        for i in range(NST):
            a = singles.tile([P, cuts[i + 1] - cuts[i]], FP, tag=f"acc{i}")
            nc.vector.memset(a[:], NEG)
            acc_st.append(a)

        ident = singles.tile([128, 128], FP, tag="ident")
        make_identity(nc, ident[:])
        eps_t = singles.tile([128, 1], FP, tag="eps")
        nc.vector.memset(eps_t[:], float(EPS))

        ar_in_all = dram.tile([B, 512], FP, name="ar_in_all", tag="ar_in_all")
        ar_out_all = dram.tile([B, 512], FP, name="ar_out_all",
                               tag="ar_out_all")

        def seg_reduce(t, la, lb, s, first):
            """max-reduce chunk-tile columns [la,lb) into segment s's slot."""
            i = bisect_right(cuts, s) - 1
            acc_col = acc_st[i][:, s - cuts[i]:s - cuts[i] + 1]
            if first:
                nc.vector.reduce_max(out=acc_col, in_=t[:, la:lb], axis=AXX)
            else:
                tmp = small.tile([P, 1], FP, tag="tmp")
                nc.vector.reduce_max(out=tmp[:], in_=t[:, la:lb], axis=AXX)
                nc.vector.tensor_max(out=acc_col, in0=acc_col, in1=tmp[:])

        def stage_tail(i):
            """fT_i = clamp(acc_i); partial W1 matmul; AllReduce stage i."""
            lo, hi = cuts[i], cuts[i + 1]
            n = hi - lo
            fTq = singles.tile([P, n], FP, tag=f"fT{i}")
            nc.vector.tensor_max(out=fTq[:], in0=acc_st[i][:],
                                 in1=bia("floor")[:, lo:hi])
            nc.sync.dma_start(out=featTout_d[:, lo:hi], in_=fTq[:])
            p1 = psum.tile([n, 512], FP, tag="p1")
            nc.tensor.matmul(p1[:], lhsT=fTq[:], rhs=w1sb,
                             start=True, stop=True)
            cp = small.tile([n, 512], FP, tag="cp")
            if i < NST - 1:
                nc.scalar.activation(out=cp[:], in_=p1[:], func=ACT_COPY)
            else:
                nc.vector.tensor_copy(out=cp[:], in_=p1[:])
            nc.sync.dma_start(out=ar_in_all[lo:hi, :], in_=cp[:])
            nc.gpsimd.collective_compute(
                "AllReduce", ADD, replica_groups=[list(range(NCORES))],
                ins=[ar_in_all[lo:hi, :].opt()],
                outs=[ar_out_all[lo:hi, :].opt()])

        def layernorm_relu(out_ap, x, g, be, n, tag):
            """out_ap = relu(LN(x) * g + be); x is (128, n) sbuf."""
            st6 = small.tile([128, 6], FP, tag=f"st6_{tag}")
            nc.vector.bn_stats(out=st6[:], in_=x[:])
            mv = small.tile([128, 2], FP, tag=f"mv_{tag}")
            nc.vector.bn_aggr(out=mv[:], in_=st6[:])
            std = small.tile([128, 1], FP, tag=f"std_{tag}")
            nc.scalar.activation(out=std[:], in_=mv[:, 1:2],
                                 func=mybir.ActivationFunctionType.Sqrt,
                                 bias=eps_t[:])
            rstd = small.tile([128, 1], FP, tag=f"rstd_{tag}")
            nc.vector.reciprocal(out=rstd[:], in_=std[:])
            xh = small.tile([128, n], FP, tag=f"xh_{tag}")
            nc.vector.tensor_scalar(out=xh[:], in0=x[:], scalar1=mv[:, 0:1],
                                    scalar2=rstd[:], op0=SUB, op1=MUL)
            if g is None:
                nc.vector.tensor_scalar_max(out=out_ap, in0=xh[:], scalar1=0.0)
            else:
                nc.vector.tensor_tensor(out=xh[:], in0=xh[:], in1=g, op=MUL)
                nc.vector.tensor_tensor(out=xh[:], in0=xh[:], in1=be, op=ADD)
                nc.vector.tensor_relu(out=out_ap, in_=xh[:])

        def mlp_block(sb):
            s0 = sb * 128
            # x = concat(relu(LN(features @ W1.T + b1)), info @ Wi.T + bi)
            x1 = small.tile([128, 512], FP, tag="x1")
            for i in range(NST):
                lo, hi = max(cuts[i], s0), min(cuts[i + 1], s0 + 128)
                if hi <= lo:
                    continue
                nc.gpsimd.dma_start(out=x1[lo - s0:hi - s0, :],
                                    in_=ar_out_all[lo:hi, :])
            nc.vector.tensor_tensor(out=x1[:], in0=x1[:], in1=b1b, op=ADD)
            x_sb = singles.tile([128, 544], FP, tag=f"x_{sb}")
            layernorm_relu(x_sb[:, 0:512], x1, g1b, be1b, 512, f"pf{sb}")

            pinf = psum.tile([128, 32], FP, tag="pinf")
            nc.tensor.matmul(pinf[:], lhsT=infoT[:, s0:s0 + 128], rhs=wit[:],
                             start=True, stop=True)
            nc.vector.tensor_tensor(out=x_sb[:, 512:544], in0=pinf[:],
                                    in1=bib, op=ADD)

            # xT chunks for the 544-contraction matmuls
            xTs = []
            for c in range(5):
                w = 128 if c < 4 else 32
                pt = psum.tile([w, 128], FP, tag="pt")
                nc.tensor.transpose(out=pt[:], in_=x_sb[:, c * 128:c * 128 + w],
                                    identity=ident[:])
                xT_c = singles.tile([w, 128], FP, tag=f"xT_{sb}_{c}")
                nc.vector.tensor_copy(out=xT_c[:], in_=pt[:])
                xTs.append(xT_c)

            def head(w1tiles, bb, g, be, w2tiles, b2b, ncols, out_dram, tag):
                ph = psum.tile([128, 256], FP, tag="ph")
                for c in range(5):
                    nc.tensor.matmul(ph[:], lhsT=xTs[c][:], rhs=w1tiles[c],
                                     start=(c == 0), stop=(c == 4))
                h = small.tile([128, 256], FP, tag=f"h_{tag}")
                nc.vector.tensor_tensor(out=h[:], in0=ph[:], in1=bb, op=ADD)
                hr = small.tile([128, 256], FP, tag=f"hr_{tag}")
                layernorm_relu(hr[:], h, g, be, 256, f"h{tag}{sb}")
                pp = psum.tile([128, 32], FP, tag="pp")
                for c in range(2):
                    pt2 = psum.tile([128, 128], FP, tag="pt")
                    nc.tensor.transpose(out=pt2[:],
                                        in_=hr[:, c * 128:(c + 1) * 128],
                                        identity=ident[:])
                    hT = small.tile([128, 128], FP, tag=f"hT_{tag}")
                    nc.vector.tensor_copy(out=hT[:], in_=pt2[:])
                    nc.tensor.matmul(pp[:, 0:ncols], lhsT=hT[:],
                                     rhs=w2tiles[c],
                                     start=(c == 0), stop=(c == 1))
                outt = small.tile([128, ncols], FP, tag=f"o_{tag}")
                nc.vector.tensor_tensor(out=outt[:], in0=pp[:, 0:ncols],
                                        in1=b2b, op=ADD)
                nc.gpsimd.dma_start(out=out_dram[s0:s0 + 128, :], in_=outt[:])

            head(wa1s, ba1b, gab, beab, wa2s, ba2b, 6, pred_d, "a")
            head(wo1s, bo1b, gob, beob, wo2s, bo2b, 18, off_d, "o")

        # number of stages fully covering each seg-block
        st_cover = [sum(1 for i in range(NST) if cuts[i + 1] <= (sb + 1) * 128)
                    for sb in range(SB)]

        # ---- stream + segment reduce + interleaved stage/MLP emission ---
        stage_i = 0
        mlp_done = 0
        for ci in range(nchunks):
            c0 = ci * cw
            c1 = min(n_points, c0 + cw)
            w = c1 - c0
            t = chunks.tile([P, cw], FP, tag="chunk")
            eng = nc.sync if ci % 2 == 0 else nc.scalar
            eng.dma_start(out=t[:, :w], in_=featT[:, c0:c1])
            if ci == 1 or (nchunks == 1 and ci == 0):
                load_consts()
            for s in range(B):
                a = max(bounds[s], c0)
                b = min(bounds[s + 1], c1)
                if b <= a:
                    continue
                seg_reduce(t, a - c0, b - c0, s, bounds[s] >= c0)
            while stage_i < NST and st_done_chunk[stage_i] == ci:
                stage_tail(stage_i)
                stage_i += 1
            # emit seg-block sb's MLP once one further stage is also done:
            # its collectives are then long finished, so the chain never
            # stalls the in-order engine queues mid-stream.
            while mlp_done < SB and stage_i >= st_cover[mlp_done] + 1:
                mlp_block(mlp_done)
                mlp_done += 1
        while stage_i < NST:    # stages with no points still need output
            stage_tail(stage_i)
            stage_i += 1
        while mlp_done < SB:
            mlp_block(mlp_done)
            mlp_done += 1

    nc.compile()
    return nc


_PROG_CACHE = {}

# test harness hooks: set TRACE=True before calling kernel() to capture an
# NTFF profile; the measured NEFF time lands in LAST_EXEC_NS.
TRACE = False
LAST_EXEC_NS = None
LAST_RESULTS = None


def _ensure_ntff_hook():
    """The image's antenv package lacks axon_hooks; synthesize it so
    run_bass_kernel_spmd(trace=True) can reach the NTFF profiler."""
    import sys
    import types
    try:
        from antenv.axon_hooks import get_axon_ntff_profile_hook  # noqa: F401
        return
    except ImportError:
        pass
    import antenv
    from trn_agent_boot.trn_boot import _ntff_profile_via_ctypes
    hook = _ntff_profile_via_ctypes("/opt/axon/libaxon_pjrt.so")
    m = types.ModuleType("antenv.axon_hooks")
    m.get_axon_ntff_profile_hook = lambda: hook
    m.set_axon_ntff_profile_hook = lambda h: None
    sys.modules["antenv.axon_hooks"] = m
    antenv.axon_hooks = m


def _get_program(bounds_t, n_points, cw, gb_id):
    key = (bounds_t, n_points, cw, gb_id)
    if key not in _PROG_CACHE:
        _PROG_CACHE[key] = _build_program(list(bounds_t), n_points, cw, gb_id)
    return _PROG_CACHE[key]


def _make_in_maps(feat, info, offsets, wd, B):
    """Build the 8 per-core input maps. wd: dict of weight arrays."""
    featT = np.ascontiguousarray(feat.T)                      # (C, N)
    w1T = np.ascontiguousarray(wd["W1"].T)                    # (C, 512)
    lens = np.diff(offsets, prepend=0)
    floor = np.where(lens < KPAD, 0.0, NEG).astype(np.float32)

    vals = dict(
        floor=floor, b1=wd["b1"], ln1g=wd["ln1_g"], ln1b=wd["ln1_b"],
        bi=wd["bi"], ba1=wd["ba1"], lnag=wd["lna_g"], lnab=wd["lna_b"],
        ba2=wd["ba2"], bo1=wd["bo1"], lnog=wd["lno_g"], lnob=wd["lno_b"],
        bo2=wd["bo2"])
    brow = np.concatenate([np.asarray(vals[k], np.float32).ravel()
                           for k, _ in _BIAS_SEGS])
    assert brow.shape[0] == NB
    biases = np.ascontiguousarray(np.broadcast_to(brow, (P, NB)))

    wa1T = np.ascontiguousarray(wd["Wa1"].T)                  # (544, 256)
    wo1T = np.ascontiguousarray(wd["Wo1"].T)
    wa2T = np.ascontiguousarray(wd["Wa2"].T)                  # (256, 6)
    wo2T = np.ascontiguousarray(wd["Wo2"].T)                  # (256, 18)
    infoT = np.zeros((4, B), np.float32)
    infoT[:3] = info.T
    wit = np.zeros((4, 32), np.float32)
    wit[:3] = wd["Wi"].T

    def wcat_for(w1t_slice):
        parts = [w1t_slice]
        parts += [wa1T[c * 128:(c + 1) * 128] for c in range(4)]
        parts += [wo1T[c * 128:(c + 1) * 128] for c in range(4)]
        parts += [wa2T[0:128], wa2T[128:256]]
        parts += [wo2T[0:128], wo2T[128:256]]
        return np.ascontiguousarray(np.concatenate(parts, axis=1))

    common = dict(
        biases=biases,
        infox=infoT, witx=wit,
        wa1x=np.ascontiguousarray(wa1T[512:544]),
        wo1x=np.ascontiguousarray(wo1T[512:544]),
    )
    common = {k: np.ascontiguousarray(v, dtype=np.float32)
              for k, v in common.items()}
    in_maps = []
    for d in range(NCORES):
        m = dict(common)
        m["featT"] = featT[d * P:(d + 1) * P]
        m["wcat"] = wcat_for(w1T[d * P:(d + 1) * P])
        in_maps.append(m)
    return in_maps


def _gb_identity(wd):
    def iden(g, b):
        return bool(np.all(np.asarray(g) == 1.0) and
                    np.all(np.asarray(b) == 0.0))
    return (iden(wd["ln1_g"], wd["ln1_b"]),
            iden(wd["lna_g"], wd["lna_b"]),
            iden(wd["lno_g"], wd["lno_b"]))


def kernel(**inputs):
    xs = {k: np.asarray(v) for k, v in inputs.items()}
    feat = np.ascontiguousarray(xs["feat"], dtype=np.float32)
    info = np.ascontiguousarray(xs["info"], dtype=np.float32)
    offsets = np.asarray(xs["offsets"]).astype(np.int64)
    n, c = feat.shape
    B = offsets.shape[0]
    assert c == NCORES * P

    bounds = np.concatenate([[0], offsets]).astype(np.int64)
    cw = 8192
    gb_id = _gb_identity(xs)
    nc = _get_program(tuple(int(v) for v in bounds), n, cw, gb_id)
    in_maps = _make_in_maps(feat, info, offsets, xs, B)

    if TRACE:
        _ensure_ntff_hook()
        import concourse.bass_utils as _bu
        _bu.upload_artifacts = lambda d: d  # no S3 in this container
    res = run_bass_kernel_spmd(nc, in_maps, core_ids=list(range(NCORES)),
                               trace=TRACE)
    global LAST_EXEC_NS, LAST_RESULTS
    LAST_EXEC_NS = res.exec_time_ns
    LAST_RESULTS = res
    featuresT = np.concatenate(
        [res.results[d]["featT_out"] for d in range(NCORES)], axis=0)  # (C, B)
    features = np.ascontiguousarray(featuresT.T)
    pred = res.results[0]["pred_out"]
    offset = res.results[0]["off_out"].reshape(B, 6, 3)
    return features, pred, offset
